# revision 3
# baseline (speedup 1.0000x reference)
"""Trainium2 Bass kernel for AttentiveNonLocalBlock2D.

Per-core SPMD over 8 NeuronCores, sequence-parallel over N=H*W with a
global pixel permutation n' = (w%8)*1152 + h*12 + w//8 so the x8 bilinear
upsample writes contiguous fp16 blocks. Core k owns permuted block k.

  Phase A: 3x stride-2 conv gating unit (fp16 PE, conv1 2-tap-paired via a
    column-shifted xpad copy in partitions 64-127) -> bilinear x8 upsample
    fused per-block with sigmoid (ACT) * x (DVE fp16 2x) -> xg16; per-block
    phi chunks (prescaled by A=2^10/ln2, psum evacuated alternately by
    ACT-Copy/DVE) and G^T = xg^T (Ww gw)^T groups; own-chunk theta late
    (dynamic slices are not dep-tracked -> must follow all writes).
  Pass 1 per m-tile (~1/3 D, 2/3 A interleaved for ACT/DVE balance):
    psum = A*f = A * phi^T theta (fp16 PE), then
    A: ACT exp(psum/A - 5) -> fp16 s-cache, Z row-sum via accum_out;
    D: DVE Schraudolph max(psum+S1,0)->int16 (bitcast fp16), Z via a
       4x-mode tensor_scalar identity pass with accum_out.
    (per-m-tile exp bias differences cancel in the n-softmax)
  Z AllReduced in 5 chunks (28/16/12/8/8 m-tiles); each chunk's pass-2
  matmuls (two interleaved psum half-chains per 512-col sub) drain at ~3
  ops per pass-1 tile in PE's slack; tail = last AR + 8-tile pass 2.
  Pass 2: out[64, n] = sum_m (G[:,m]/Z[m]) s[m,n] + residual xg.
"""

import sys

if "/opt/trn_rl_repo" not in sys.path:
    sys.path.insert(0, "/opt/trn_rl_repo")

import numpy as np
import os

NCORES = 8
C, CI, H, W = 64, 32, 96, 96
N = H * W            # 9216
CH = N // NCORES     # 1152 pixels per core (one w%8 phase)
MT = N // 128        # 72 m-tiles
SUBS = ((0, 512), (512, 512), (1024, 128))
DMOD = int(os.environ.get("DMOD", "3"))
PACE = int(os.environ.get("PACE", "3"))
NA = 24              # m-tiles in the early s-cache pool
ARC = tuple(int(x) for x in
            os.environ.get("ARC", "28,16,12,8,8").split(","))

A_EXP = float(2.0**10 / np.log(2.0))   # fp16 Schraudolph scale
B_SCH = 5.94                            # exp bias for schraudolph tiles
C_SCH = -189.0                          # truncation-centering correction
S1_SCH = float(15360.0 + C_SCH - A_EXP * B_SCH)
EXP_BIAS = -5.0                         # bias for ACT tiles


# 8 early DVE-schraudolph tiles run during phase A2 (no ACT exp allowed
# before the last sigmoid); later tiles are classed by emission order
EARLY_D = (1, 4, 7, 10, 13, 16, 19, 22)


_compiled = {}


def _build(single=False, debug=False):
    import concourse.bacc as bacc
    import concourse.bass as bass
    import concourse.mybir as mybir
    import concourse.tile as tile

    f16 = mybir.dt.float16
    f32 = mybir.dt.float32
    i16 = mybir.dt.int16
    AF = mybir.ActivationFunctionType
    ALU = mybir.AluOpType
    X = mybir.AxisListType.X

    nc = bacc.Bacc("TRN2", target_bir_lowering=False, debug=False,
                   num_devices=1 if single else NCORES)

    xpad_io = nc.dram_tensor("xpad", [C, 98, 98], f16, kind="ExternalInput")
    xpadB_io = nc.dram_tensor("xpadB", [C, 98, 98], f16, kind="ExternalInput")
    w1p_io = nc.dram_tensor("w1p", [2 * C, 3 * C], f16, kind="ExternalInput")
    w1s_io = nc.dram_tensor("w1s", [C, 3 * C], f16, kind="ExternalInput")
    x16_io = nc.dram_tensor("x16", [C, N], f16, kind="ExternalInput")
    w1_io = nc.dram_tensor("w1", [C, 9 * C], f16, kind="ExternalInput")
    w2_io = nc.dram_tensor("w2", [C, 9 * C], f16, kind="ExternalInput")
    w3_io = nc.dram_tensor("w3", [C, 9 * C], f16, kind="ExternalInput")
    twT_io = nc.dram_tensor("twT", [C, CI], f16, kind="ExternalInput")
    pwTA_io = nc.dram_tensor("pwTA", [C, CI], f16, kind="ExternalInput")
    ET_io = nc.dram_tensor("ET", [C, C], f16, kind="ExternalInput")
    out_io = nc.dram_tensor("out", [C, CH], f32, kind="ExternalOutput")

    with tile.TileContext(nc) as tc:
        with tc.tile_pool(name="persist", bufs=1) as pp, \
             tc.tile_pool(name="dram", bufs=1, space="DRAM") as dp:
            zsum = pp.tile([128, MT], f32)
            nb5 = pp.tile([128, 1], f32)
            nc.gpsimd.memset(nb5[:], EXP_BIAS)
            zin = []
            zout = []
            for ci, w in enumerate(ARC):
                zin.append(dp.tile([128, w], f32, name=f"zin{ci}"))
                zout.append(dp.tile([128, w], f32, addr_space="Shared",
                                    name=f"zout{ci}"))

            with tc.tile_pool(name="hand", bufs=1) as hp, \
                 tc.tile_pool(name="scA", bufs=1) as scpA, \
                 tc.tile_pool(name="p1ps", bufs=2, space="PSUM") as p1ps:
                phiA = hp.tile([CI, N], f16)          # A * phi
                thc = hp.tile([CI, CH], f16)
                G16 = hp.tile([128, MT * C], f16)
                G3 = G16[:].rearrange("p (j c) -> p j c", c=C)
                xgc = hp.tile([C, CH], f16)           # own-chunk x_gated
                outsb = hp.tile([C, CH], f32)
                s_cacheA = scpA.tile([128, NA * CH], f16)

                def s_sl(j):
                    if j < NA:
                        return s_cacheA[:, j * CH:(j + 1) * CH]
                    r = MT - 1 - j
                    return s_cacheB[:, r * CH:(r + 1) * CH]

                def pass1_tile(j, k):
                    fps = p1ps.tile([128, CH], f32, tag="fps", name="fps")
                    for o0, w in SUBS:
                        nc.tensor.matmul(fps[:, o0:o0 + w],
                                         phiA[:, j * 128:(j + 1) * 128],
                                         thc[:, o0:o0 + w],
                                         start=True, stop=True)
                    ssl = s_sl(j)
                    if k == "A":
                        nc.scalar.activation(ssl, fps[:], AF.Exp,
                                             bias=nb5[:],
                                             scale=float(1.0 / A_EXP),
                                             accum_out=zsum[:, j:j + 1])
                    else:
                        nc.vector.tensor_scalar(ssl.bitcast(i16), fps[:],
                                                S1_SCH, 0.0,
                                                op0=ALU.add, op1=ALU.max)
                        # Z row-sum via 4x-mode identity pass with accum
                        nc.vector.tensor_scalar(ssl, ssl, 0.0, None,
                                                op0=ALU.add, op1=ALU.add,
                                                accum_out=zsum[:, j:j + 1])

                # ==================== PHASE A ====================
                with tc.tile_pool(name="mid", bufs=1) as pm, \
                     tc.tile_pool(name="pup", bufs=1) as pu:
                    yh = pm.tile([C, N], f16)     # permuted upsampled logits
                    xg16 = pm.tile([C, N], f16)   # permuted x_gated
                    y3v = pu.tile([C, 14, 12], f32)
                    yvp = pu.tile([C, 96, 14], f16)
                    dv = pu.tile([C, 13, 12], f32)
                    dh = pu.tile([C, 96, 13], f16)

                    # --- A1: convs (pool closes before A2 to free SBUF) ---
                    with tc.tile_pool(name="pcv", bufs=1) as pa:
                        w1psb = pa.tile([2 * C, 3 * C], f16)
                        nc.sync.dma_start(w1psb[:], w1p_io[:])
                        w1ssb = pa.tile([C, 3 * C], f16)
                        nc.sync.dma_start(w1ssb[:], w1s_io[:])
                        # xp2: partitions 0-63 = xpad, 64-127 = xpad shifted
                        # one column left, so one matmul covers 2 taps; the
                        # shifted copy loads via the gpsimd software DGE in
                        # parallel with the sync HWDGE stream
                        xp2 = pa.tile([2 * C, 98, 98], f16)
                        for b in range(4):
                            r0, r1 = 26 * b, min(26 * b + 26, 98)
                            nc.sync.dma_start(xp2[0:C, r0:r1, :],
                                              xpad_io[:, r0:r1, :])
                            nc.gpsimd.dma_start(xp2[C:2 * C, r0:r1, :],
                                                xpadB_io[:, r0:r1, :])
                        w2sb = pa.tile([C, 9 * C], f16)
                        nc.sync.dma_start(w2sb[:], w2_io[:])
                        w3sb = pa.tile([C, 9 * C], f16)
                        nc.sync.dma_start(w3sb[:], w3_io[:])
                        x16sb = pm.tile([C, N], f16)
                        nc.sync.dma_start(x16sb[:, 0:4608], x16_io[:, 0:4608])
                        nc.sync.dma_start(x16sb[:, 4608:N], x16_io[:, 4608:N])
                        twTsb = pm.tile([C, CI], f16)
                        nc.sync.dma_start(twTsb[:], twT_io[:])
                        pwTsb = pm.tile([C, CI], f16)
                        nc.sync.dma_start(pwTsb[:], pwTA_io[:])
                        ETsb = pm.tile([C, C], f16)
                        nc.sync.dma_start(ETsb[:], ET_io[:])

                        with tc.tile_pool(name="cvps", bufs=2,
                                          space="PSUM") as paps:
                            # conv1: 96x96 -> 48x48, stride 2, pad 1, lrelu
                            y1p = pa.tile([C, 50, 50], f16)
                            nc.gpsimd.memset(y1p[:], 0.0)
                            for g in range(6):
                                ps1 = paps.tile([C, 8, 48], f32, tag="cv",
                                                name="ps1")
                                # singles (xpad only) first: they don't wait
                                # for the software-DGE xpadB stream
                                for dy in range(3):
                                    nc.tensor.matmul(
                                        ps1[:], w1ssb[:, dy * C:(dy + 1) * C],
                                        xp2[0:C,
                                            16 * g + dy: 16 * g + dy + 16: 2,
                                            2: 98: 2],
                                        start=(dy == 0), stop=False)
                                for dy in range(3):
                                    nc.tensor.matmul(
                                        ps1[:], w1psb[:, dy * C:(dy + 1) * C],
                                        xp2[:, 16 * g + dy: 16 * g + dy + 16: 2,
                                            0: 96: 2],
                                        start=False, stop=(dy == 2))
                                # lrelu(x) = 0.6*x + 0.4*|x| (Abs is in
                                # every act table -> no table reloads)
                                ab1 = pa.tile([C, 8 * 48], f32, tag="ab",
                                              name="ab1", bufs=2)
                                nc.scalar.activation(ab1[:], ps1[:], AF.Abs,
                                                     scale=0.4)
                                nc.vector.scalar_tensor_tensor(
                                    y1p[:, 1 + 8 * g: 9 + 8 * g, 1:49],
                                    ps1[:], 0.6, ab1[:],
                                    op0=ALU.mult, op1=ALU.add)

                            # conv2: 48x48 -> 24x24
                            y2p = pa.tile([C, 26, 26], f16)
                            nc.gpsimd.memset(y2p[:], 0.0)
                            for g in range(2):
                                ps2 = paps.tile([C, 12, 24], f32, tag="cv",
                                                name="ps2")
                                for t in range(9):
                                    dy, dx = t // 3, t % 3
                                    nc.tensor.matmul(
                                        ps2[:], w2sb[:, t * C:(t + 1) * C],
                                        y1p[:, 24 * g + dy: 24 * g + dy + 24: 2,
                                            dx: dx + 48: 2],
                                        start=(t == 0), stop=(t == 8))
                                ab2 = pa.tile([C, 12 * 24], f32, tag="ab",
                                              name="ab2", bufs=2)
                                nc.scalar.activation(ab2[:], ps2[:], AF.Abs,
                                                     scale=0.4)
                                nc.vector.scalar_tensor_tensor(
                                    y2p[:, 1 + 12 * g: 13 + 12 * g, 1:25],
                                    ps2[:], 0.6, ab2[:],
                                    op0=ALU.mult, op1=ALU.add)

                            # conv3: 24x24 -> 12x12 (no activation)
                            ps3 = paps.tile([C, 12, 12], f32, tag="cv",
                                            name="ps3")
                            for t in range(9):
                                dy, dx = t // 3, t % 3
                                nc.tensor.matmul(
                                    ps3[:], w3sb[:, t * C:(t + 1) * C],
                                    y2p[:, dy: dy + 24: 2, dx: dx + 24: 2],
                                    start=(t == 0), stop=(t == 8))
                            nc.vector.tensor_copy(y3v[:, 1:13, :], ps3[:])
                            nc.vector.tensor_copy(y3v[:, 0:1, :],
                                                  ps3[:, 0:1, :])
                            nc.vector.tensor_copy(y3v[:, 13:14, :],
                                                  ps3[:, 11:12, :])

                    # --- A2: upsample blocks fused with gate/projections ---
                    with tc.tile_pool(name="pa2", bufs=1) as pa, \
                         tc.tile_pool(name="paps2", bufs=1,
                                      space="PSUM") as paps:
                        # bilinear x8 vertical (tiny DVE ops)
                        nc.vector.tensor_sub(dv[:], y3v[:, 1:14, :],
                                             y3v[:, 0:13, :])
                        for r in range(8):
                            t = (r + 0.5) / 8 - 0.5
                            kr, b = (0, 1 + t) if r < 4 else (1, t)
                            nc.vector.scalar_tensor_tensor(
                                yvp[:, r:96:8, 1:13], dv[:, kr:kr + 12, :],
                                float(b), y3v[:, kr:kr + 12, :],
                                op0=ALU.mult, op1=ALU.add)
                        nc.vector.tensor_copy(yvp[:, :, 0:1], yvp[:, :, 1:2])
                        nc.vector.tensor_copy(yvp[:, :, 13:14],
                                              yvp[:, :, 12:13])
                        yh3 = yh[:].rearrange("c (r n) -> c r n", r=8)
                        nc.vector.tensor_sub(dh[:], yvp[:, :, 1:14],
                                             yvp[:, :, 0:13])

                        def phi_chunk(ci):
                            sl = slice(ci * 512, (ci + 1) * 512)
                            pps = paps.tile([CI, 512], f32,
                                            tag="pj", name="pps")
                            nc.tensor.matmul(pps[:], pwTsb[:], xg16[:, sl],
                                             start=True, stop=True)
                            # alternate evacuation engine so neither ACT nor
                            # DVE backlog stalls the single-bank psum chain
                            if ci % 2 == 0:
                                nc.scalar.activation(phiA[:, sl], pps[:],
                                                     AF.Copy)
                            else:
                                nc.vector.tensor_copy(phiA[:, sl], pps[:])

                        def gt_group(gg):
                            gps = paps.tile([128, 8 * C], f32, tag="gps",
                                            name="gps")
                            for u in range(8):
                                j = gg * 8 + u
                                nc.tensor.matmul(
                                    gps[:, u * C:(u + 1) * C],
                                    xg16[:, j * 128:(j + 1) * 128],
                                    ETsb[:], start=True, stop=True)
                            nc.vector.tensor_copy(
                                G16[:, gg * 8 * C:(gg + 1) * 8 * C], gps[:])

                        # per block: horizontal upsample -> sigmoid -> gate
                        # mul -> phi chunks / G^T groups / early pass-1
                        # tiles, all pipelined per-block so no engine waits
                        # for the whole upsample. Sigmoid-vs-exp act tables:
                        # all sigmoids are queued before the first exp.
                        cdone = gdone = tdone = 0
                        for r in range(8):
                            t = (r + 0.5) / 8 - 0.5
                            kr, b = (0, 1 + t) if r < 4 else (1, t)
                            out3 = yh3[:, r, :].rearrange("c (h j) -> c h j",
                                                          j=12)
                            dhs = pa.tile([C, 96, 12], f16, tag="dhs",
                                          name="dhs", bufs=3)
                            nc.vector.tensor_scalar(
                                dhs[:], dh[:, :, kr:kr + 12],
                                float(b), None, op0=ALU.mult)
                            eng = nc.vector if r < 6 else nc.gpsimd
                            eng.tensor_add(out3, dhs[:],
                                           yvp[:, :, kr:kr + 12])

                            sl = slice(r * CH, (r + 1) * CH)
                            gt = pa.tile([C, CH], f16, tag="gt", name="gt",
                                         bufs=3)
                            nc.scalar.activation(gt[:], yh[:, sl], AF.Sigmoid)
                            nc.vector.tensor_mul(xg16[:, sl], gt[:],
                                                 x16sb[:, sl])
                            if r == 0:
                                # own-chunk early path: dynamic slices wait
                                # on the full yh/xg16 and park in the 4-deep
                                # wait queues while later blocks flow past
                                yhc = pa.tile([C, CH], f16)
                                xc16 = pa.tile([C, CH], f16)
                                with tc.tile_critical():
                                    pid = nc.vector.partition_id()
                                    col0 = pid * CH
                                    nc.vector.tensor_copy(
                                        yhc[:], yh[:, bass.ds(col0, CH)])
                                    nc.vector.tensor_copy(
                                        xc16[:], x16sb[:, bass.ds(col0, CH)])
                                gtc = pa.tile([C, CH], f16)
                                nc.scalar.activation(gtc[:], yhc[:],
                                                     AF.Sigmoid)
                                nc.vector.tensor_mul(xgc[:], gtc[:], xc16[:])
                                for si, (o0, w) in enumerate(SUBS):
                                    tps = paps.tile([CI, 512], f32,
                                                    tag="pj", name="tps")
                                    nc.tensor.matmul(tps[:, 0:w], twTsb[:],
                                                     xgc[:, o0:o0 + w],
                                                     start=True, stop=True)
                                    nc.vector.tensor_copy(thc[:, o0:o0 + w],
                                                          tps[:, 0:w])
                            while (cdone + 1) * 512 <= (r + 1) * CH:
                                phi_chunk(cdone)
                                cdone += 1
                                if (gdone + 1) * 1024 <= (r + 1) * CH:
                                    gt_group(gdone)
                                    gdone += 1
                                # early pass-1 tiles once theta is ready
                                # (~chunk 11): only DVE-schraudolph tiles may
                                # run before the last sigmoid is queued (ACT
                                # exp would force an act-table reload)
                                while cdone >= 11 and tdone < len(EARLY_D) \
                                        and EARLY_D[tdone] <= \
                                        4 * (cdone - 1):
                                    pass1_tile(EARLY_D[tdone], "D")
                                    tdone += 1
                        while gdone < 9:
                            gt_group(gdone)
                            gdone += 1
                        while tdone < len(EARLY_D):
                            pass1_tile(EARLY_D[tdone], "D")
                            tdone += 1

                if debug:
                    dbg_th = nc.dram_tensor("dbg_th", [CI, CH], f16,
                                            kind="ExternalOutput")
                    dbg_s0 = nc.dram_tensor("dbg_s0", [128, 2 * CH], f16,
                                            kind="ExternalOutput")
                    nc.sync.dma_start(dbg_th[:], thc[:])
                    nc.sync.dma_start(dbg_s0[:, 0:CH], s_sl(0))
                    nc.sync.dma_start(dbg_s0[:, CH:2 * CH], s_sl(1))

                # ============ PASS 1 + interleaved PASS 2 chunks ============
                with tc.tile_pool(name="scache", bufs=1) as scp:
                    s_cacheB = scp.tile([128, (MT - NA) * CH], f16)

                    def allreduce(ci, j0, j1):
                        nc.sync.dma_start(zin[ci][:], zsum[:, j0:j1])
                        if single:
                            nc.sync.dma_start(zout[ci][:], zin[ci][:])
                        else:
                            nc.gpsimd.collective_compute(
                                "AllReduce", ALU.add,
                                replica_groups=[list(range(NCORES))],
                                ins=[zin[ci].opt()], outs=[zout[ci].opt()])

                    with tc.tile_pool(name="p2", bufs=1) as p2, \
                         tc.tile_pool(name="p2ps", bufs=2,
                                      space="PSUM") as p2ps:

                        def scale_G(ci, j0, j1):
                            w = j1 - j0
                            zf = p2.tile([128, ARC[0]], f32, tag="zf",
                                         name="zf", bufs=2)
                            nc.sync.dma_start(zf[:, 0:w], zout[ci][:])
                            rz = p2.tile([128, ARC[0]], f16, tag="rz",
                                         name="rz", bufs=2)
                            with nc.allow_low_precision(
                                    reason="1/Z weights tolerate fp16"):
                                nc.vector.reciprocal(rz[:, 0:w],
                                                     zf[:, 0:w])
                            rzb = rz[:, 0:w].unsqueeze(-1).to_broadcast(
                                (128, w, C))
                            # last chunk's scale is on the serial tail: DVE
                            eng = nc.vector if j1 == MT else nc.gpsimd
                            eng.tensor_mul(G3[:, j0:j1, :],
                                           G3[:, j0:j1, :], rzb)

                        def pass2_ops(ci, j0, j1, first, last):
                            # two interleaved half-chains per sub (both psum
                            # bufs) hide the serial accumulation latency
                            nt = j1 - j0
                            h0 = nt // 2
                            for o0, w in SUBS:
                                ypsA = p2ps.tile([64, 512], f32, tag="yps",
                                                 name="ypsA")
                                ypsB = p2ps.tile([64, 512], f32, tag="yps",
                                                 name="ypsB")
                                def mk(yps, jj, u, lu, o0=o0, w=w):
                                    return lambda: nc.tensor.matmul(
                                        yps[:, 0:w], G3[:, jj, :],
                                        s_sl(jj)[:, o0:o0 + w],
                                        start=(u == 0), stop=(u == lu),
                                        skip_group_check=True)
                                for u in range(h0):
                                    yield mk(ypsA, j0 + u, u, h0 - 1)
                                    yield mk(ypsB, j0 + h0 + u, u,
                                             nt - h0 - 1)
                                if nt - h0 > h0:
                                    yield mk(ypsB, j1 - 1, nt - h0 - 1,
                                             nt - h0 - 1)
                                if first:
                                    yield lambda yps=ypsA, o0=o0, w=w: \
                                        nc.vector.tensor_add(
                                            outsb[:, o0:o0 + w],
                                            yps[:, 0:w], xgc[:, o0:o0 + w])
                                else:
                                    yield lambda yps=ypsA, o0=o0, w=w: \
                                        nc.vector.tensor_add(
                                            outsb[:, o0:o0 + w],
                                            outsb[:, o0:o0 + w],
                                            yps[:, 0:w])
                                if last:
                                    def fin(yps=ypsB, o0=o0, w=w):
                                        nc.vector.tensor_add(
                                            outsb[:, o0:o0 + w],
                                            outsb[:, o0:o0 + w],
                                            yps[:, 0:w])
                                        nc.sync.dma_start(
                                            out_io[:, o0:o0 + w],
                                            outsb[:, o0:o0 + w])
                                    yield fin
                                else:
                                    yield lambda yps=ypsB, o0=o0, w=w: \
                                        nc.vector.tensor_add(
                                            outsb[:, o0:o0 + w],
                                            outsb[:, o0:o0 + w],
                                            yps[:, 0:w])

                        # chunked AR/pass-2 pipeline: AR(ci) fires right
                        # after its last tile; its pass-2 starts 3 tiles
                        # later (AR latency cover) and drains at ~5 ops per
                        # pass-1 tile so PE never starves the exp engines
                        NC_ = len(ARC)
                        B = [0]
                        for w in ARC:
                            B.append(B[-1] + w)
                        rest = [j for j in range(MT) if j not in EARLY_D]
                        # AR ci can fire once all tiles < B[ci+1] are done
                        arpt = {}
                        seen = set(EARLY_D)
                        for i, j in enumerate(rest):
                            seen.add(j)
                            for ci in range(NC_):
                                if B[ci + 1] - 1 in seen and \
                                        all(x in seen
                                            for x in range(B[ci + 1])):
                                    if ci not in arpt.values():
                                        arpt[i] = ci
                        scale_at = {}
                        gen_at = {}
                        for i, ci in arpt.items():
                            scale_at.setdefault(i + 1, ci)
                            gen_at.setdefault(min(i + 9, len(rest) - 3), ci)
                        active = []
                        for i, j in enumerate(rest):
                            kls = "D" if i % 7 in (2, 5) else "A"
                            pass1_tile(j, kls)
                            if i in arpt:
                                ci = arpt[i]
                                allreduce(ci, B[ci], B[ci + 1])
                            if i in scale_at and scale_at[i] != NC_ - 1:
                                ci = scale_at[i]
                                scale_G(ci, B[ci], B[ci + 1])
                            if i in gen_at and gen_at[i] != NC_ - 1:
                                ci = gen_at[i]
                                active.append(pass2_ops(
                                    ci, B[ci], B[ci + 1],
                                    first=(ci == 0), last=False))
                            done = 0
                            while active and done < PACE:
                                op = next(active[0], None)
                                if op is None:
                                    active.pop(0)
                                else:
                                    op()
                                    done += 1
                        for gen in active:
                            for op in gen:
                                op()
                        ci = NC_ - 1
                        scale_G(ci, B[ci], B[ci + 1])
                        for op in pass2_ops(ci, B[ci], B[ci + 1],
                                            first=False, last=True):
                            op()

    nc.compile()
    return nc


def get_program():
    if "nc" not in _compiled:
        _compiled["nc"] = _build()
    return _compiled["nc"]


def make_in_maps(inputs):
    f16 = np.float16
    x = np.asarray(inputs["x"], np.float32).reshape(C, H, W)
    xpad = np.zeros((C, 98, 98), f16)
    xpad[:, 1:97, 1:97] = x.astype(f16)
    # permuted layout: n' = (w%8)*1152 + h*12 + w//8
    x16p = np.ascontiguousarray(
        x.reshape(C, H, W // 8, 8).transpose(0, 3, 1, 2).reshape(C, N)
    ).astype(f16)

    def conv_w(w):
        return np.ascontiguousarray(
            np.asarray(w, np.float32).transpose(1, 2, 3, 0).reshape(C, 9 * C)
        ).astype(f16)

    # xpadB = xpad shifted one column left (for the 2-tap paired conv1)
    xpadB = np.zeros((C, 98, 98), f16)
    xpadB[:, :, 0:97] = xpad[:, :, 1:98]
    # paired conv1 weights: w1p rows 0-63 tap (dy,0), rows 64-127 tap (dy,1)
    w1f = np.asarray(inputs["d1_w"], np.float32).transpose(1, 2, 3, 0)
    w1p = np.concatenate([
        np.concatenate([w1f[:, dy, 0, :], w1f[:, dy, 1, :]], axis=0)
        for dy in range(3)], axis=1).astype(f16)
    w1s = np.concatenate([w1f[:, dy, 2, :] for dy in range(3)],
                         axis=1).astype(f16)

    gw = np.asarray(inputs["g_w"], np.float32)[:, :, 0, 0]
    Ww = np.asarray(inputs["W_w"], np.float32)[:, :, 0, 0]
    base = {
        "xpad": xpad,
        "xpadB": xpadB,
        "w1p": np.ascontiguousarray(w1p),
        "w1s": np.ascontiguousarray(w1s),
        "x16": x16p,
        "w1": conv_w(inputs["d1_w"]),
        "w2": conv_w(inputs["d2_w"]),
        "w3": conv_w(inputs["d3_w"]),
        "twT": np.ascontiguousarray(
            np.asarray(inputs["th_w"], np.float32)[:, :, 0, 0].T).astype(f16),
        "pwTA": np.ascontiguousarray(
            np.asarray(inputs["ph_w"], np.float32)[:, :, 0, 0].T
            * A_EXP).astype(f16),
        "ET": np.ascontiguousarray((Ww @ gw).T).astype(f16),
    }
    return [dict(base) for _ in range(NCORES)]


def kernel(**inputs):
    from concourse import bass_utils

    nc = get_program()
    in_maps = make_in_maps(inputs)
    res = bass_utils.run_bass_kernel_spmd(nc, in_maps,
                                          core_ids=list(range(NCORES)))
    # gather permuted chunks -> full permuted [C, N] -> un-permute
    outp = np.concatenate([res.results[k]["out"] for k in range(NCORES)],
                          axis=1)
    out = outp.reshape(C, 8, H, W // 8).transpose(0, 2, 3, 1).reshape(C, H, W)
    return out.reshape(1, C, H, W).astype(np.float32)


# revision 5
# speedup vs baseline: 1.0248x; 1.0248x over previous
"""Trainium2 Bass kernel for AttentiveNonLocalBlock2D (v3).

Per-core SPMD over 8 NeuronCores, sequence-parallel over N=H*W with a
global pixel permutation n' = (w%8)*1152 + h*12 + w//8 so the x8 bilinear
upsample writes contiguous blocks. Core k owns permuted block k (w%8==k).

  Phase A: 3x stride-2 conv gating unit (fp16 PE) -> bilinear x8 upsample
    (fp16 2x STTs) -> per-block sigmoid (ACT) * x (DVE fp16 2x) -> xg16;
    phi cache prescaled by A=2^10/ln2; theta (own chunk); G^T fp16.
  Pass 1 per m-tile: psum = A*f = A * phi^T theta (fp16 PE), then one of
    A: ACT exp(psum/A - 5) -> fp16 s-cache, Z via accum_out;
    D: DVE Schraudolph max(psum+S1,0)->int16 (bitcast fp16), DVE Z-reduce;
    P: ACT Copy psum->fp16, gpsimd Schraudolph in-place, DVE Z-reduce.
    (per-m-tile exp bias differences cancel in the n-softmax)
  Z AllReduced in 3 chunks (36/24/12 m-tiles); pass-2 chunks interleaved
  under pass 1; short tail = last AR + 12-tile pass 2.
  Pass 2: out[64, n] = sum_m (G[:,m]/Z[m]) s[m,n], one psum chain per
    512-col sub per chunk, + residual xg.
"""

import sys

if "/opt/trn_rl_repo" not in sys.path:
    sys.path.insert(0, "/opt/trn_rl_repo")

import numpy as np
import os

NCORES = 8
C, CI, H, W = 64, 32, 96, 96
N = H * W            # 9216
CH = N // NCORES     # 1152 pixels per core (one w%8 phase)
MT = N // 128        # 72 m-tiles
SUBS = ((0, 512), (512, 512), (1024, 128))
DMOD = int(os.environ.get("DMOD", "3"))
PACE = int(os.environ.get("PACE", "3"))
NA = 24              # m-tiles in the early s-cache pool
ARC = tuple(int(x) for x in
            os.environ.get("ARC", "24,16,12,12,8").split(","))

A_EXP = float(2.0**10 / np.log(2.0))   # fp16 Schraudolph scale
B_SCH = 5.94                            # exp bias for schraudolph tiles
C_SCH = -189.0                          # truncation-centering correction
S1_SCH = float(15360.0 + C_SCH - A_EXP * B_SCH)
EXP_BIAS = -5.0                         # bias for ACT tiles


# 8 early DVE-schraudolph tiles run during phase A2 (no ACT exp allowed
# before the last sigmoid); later tiles are classed by emission order
EARLY_D = (1, 4, 7, 10, 13, 16, 19, 22)


_compiled = {}


def _build(single=False, debug=False):
    import concourse.bacc as bacc
    import concourse.bass as bass
    import concourse.mybir as mybir
    import concourse.tile as tile

    f16 = mybir.dt.float16
    f32 = mybir.dt.float32
    i16 = mybir.dt.int16
    AF = mybir.ActivationFunctionType
    ALU = mybir.AluOpType
    X = mybir.AxisListType.X

    nc = bacc.Bacc("TRN2", target_bir_lowering=False, debug=False,
                   num_devices=1 if single else NCORES)

    xpad_io = nc.dram_tensor("xpad", [C, 98, 98], f16, kind="ExternalInput")
    xpadB_io = nc.dram_tensor("xpadB", [C, 98, 98], f16, kind="ExternalInput")
    w1p_io = nc.dram_tensor("w1p", [2 * C, 3 * C], f16, kind="ExternalInput")
    w1s_io = nc.dram_tensor("w1s", [C, 3 * C], f16, kind="ExternalInput")
    x16_io = nc.dram_tensor("x16", [C, N], f16, kind="ExternalInput")
    w1_io = nc.dram_tensor("w1", [C, 9 * C], f16, kind="ExternalInput")
    w2_io = nc.dram_tensor("w2", [C, 9 * C], f16, kind="ExternalInput")
    w3_io = nc.dram_tensor("w3", [C, 9 * C], f16, kind="ExternalInput")
    twT_io = nc.dram_tensor("twT", [C, CI], f16, kind="ExternalInput")
    pwTA_io = nc.dram_tensor("pwTA", [C, CI], f16, kind="ExternalInput")
    ET_io = nc.dram_tensor("ET", [C, C], f16, kind="ExternalInput")
    out_io = nc.dram_tensor("out", [C, CH], f32, kind="ExternalOutput")

    with tile.TileContext(nc) as tc:
        with tc.tile_pool(name="persist", bufs=1) as pp, \
             tc.tile_pool(name="dram", bufs=1, space="DRAM") as dp:
            zsum = pp.tile([128, MT], f32)
            nb5 = pp.tile([128, 1], f32)
            nc.gpsimd.memset(nb5[:], EXP_BIAS)
            zin = []
            zout = []
            for ci, w in enumerate(ARC):
                zin.append(dp.tile([128, w], f32, name=f"zin{ci}"))
                zout.append(dp.tile([128, w], f32, addr_space="Shared",
                                    name=f"zout{ci}"))

            with tc.tile_pool(name="hand", bufs=1) as hp, \
                 tc.tile_pool(name="scA", bufs=1) as scpA, \
                 tc.tile_pool(name="p1ps", bufs=2, space="PSUM") as p1ps:
                phiA = hp.tile([CI, N], f16)          # A * phi
                thc = hp.tile([CI, CH], f16)
                G16 = hp.tile([128, MT * C], f16)
                G3 = G16[:].rearrange("p (j c) -> p j c", c=C)
                xgc = hp.tile([C, CH], f16)           # own-chunk x_gated
                outsb = hp.tile([C, CH], f32)
                s_cacheA = scpA.tile([128, NA * CH], f16)

                def s_sl(j):
                    if j < NA:
                        return s_cacheA[:, j * CH:(j + 1) * CH]
                    r = MT - 1 - j
                    return s_cacheB[:, r * CH:(r + 1) * CH]

                def pass1_tile(j, k):
                    fps = p1ps.tile([128, CH], f32, tag="fps", name="fps")
                    for o0, w in SUBS:
                        nc.tensor.matmul(fps[:, o0:o0 + w],
                                         phiA[:, j * 128:(j + 1) * 128],
                                         thc[:, o0:o0 + w],
                                         start=True, stop=True)
                    ssl = s_sl(j)
                    if k == "A":
                        nc.scalar.activation(ssl, fps[:], AF.Exp,
                                             bias=nb5[:],
                                             scale=float(1.0 / A_EXP),
                                             accum_out=zsum[:, j:j + 1])
                    else:
                        nc.vector.tensor_scalar(ssl.bitcast(i16), fps[:],
                                                S1_SCH, 0.0,
                                                op0=ALU.add, op1=ALU.max)
                        # Z row-sum via 4x-mode identity pass with accum
                        nc.vector.tensor_scalar(ssl, ssl, 0.0, None,
                                                op0=ALU.add, op1=ALU.add,
                                                accum_out=zsum[:, j:j + 1])

                # ==================== PHASE A ====================
                with tc.tile_pool(name="mid", bufs=1) as pm, \
                     tc.tile_pool(name="pup", bufs=1) as pu:
                    yh = pm.tile([C, N], f16)     # permuted upsampled logits
                    xg16 = pm.tile([C, N], f16)   # permuted x_gated
                    y3v = pu.tile([C, 14, 12], f32)
                    yvp = pu.tile([C, 96, 14], f16)
                    dv = pu.tile([C, 13, 12], f32)
                    dh = pu.tile([C, 96, 13], f16)

                    # --- A1: convs (pool closes before A2 to free SBUF) ---
                    with tc.tile_pool(name="pcv", bufs=1) as pa:
                        w1psb = pa.tile([2 * C, 3 * C], f16)
                        nc.sync.dma_start(w1psb[:], w1p_io[:])
                        w1ssb = pa.tile([C, 3 * C], f16)
                        nc.sync.dma_start(w1ssb[:], w1s_io[:])
                        # xp2: partitions 0-63 = xpad, 64-127 = xpad shifted
                        # one column left, so one matmul covers 2 taps; the
                        # shifted copy loads via the gpsimd software DGE in
                        # parallel with the sync HWDGE stream
                        xp2 = pa.tile([2 * C, 98, 98], f16)
                        for b in range(4):
                            r0, r1 = 26 * b, min(26 * b + 26, 98)
                            nc.sync.dma_start(xp2[0:C, r0:r1, :],
                                              xpad_io[:, r0:r1, :])
                            nc.gpsimd.dma_start(xp2[C:2 * C, r0:r1, :],
                                                xpadB_io[:, r0:r1, :])
                        w2sb = pa.tile([C, 9 * C], f16)
                        nc.sync.dma_start(w2sb[:], w2_io[:])
                        w3sb = pa.tile([C, 9 * C], f16)
                        nc.sync.dma_start(w3sb[:], w3_io[:])
                        x16sb = pm.tile([C, N], f16)
                        nc.sync.dma_start(x16sb[:, 0:4608], x16_io[:, 0:4608])
                        nc.sync.dma_start(x16sb[:, 4608:N], x16_io[:, 4608:N])
                        twTsb = pm.tile([C, CI], f16)
                        nc.sync.dma_start(twTsb[:], twT_io[:])
                        pwTsb = pm.tile([C, CI], f16)
                        nc.sync.dma_start(pwTsb[:], pwTA_io[:])
                        ETsb = pm.tile([C, C], f16)
                        nc.sync.dma_start(ETsb[:], ET_io[:])

                        with tc.tile_pool(name="cvps", bufs=2,
                                          space="PSUM") as paps:
                            # conv1: 96x96 -> 48x48, stride 2, pad 1, lrelu
                            y1p = pa.tile([C, 50, 50], f16)
                            nc.gpsimd.memset(y1p[:], 0.0)
                            for g in range(6):
                                ps1 = paps.tile([C, 8, 48], f32, tag="cv",
                                                name="ps1")
                                # singles (xpad only) first: they don't wait
                                # for the software-DGE xpadB stream
                                for dy in range(3):
                                    nc.tensor.matmul(
                                        ps1[:], w1ssb[:, dy * C:(dy + 1) * C],
                                        xp2[0:C,
                                            16 * g + dy: 16 * g + dy + 16: 2,
                                            2: 98: 2],
                                        start=(dy == 0), stop=False)
                                for dy in range(3):
                                    nc.tensor.matmul(
                                        ps1[:], w1psb[:, dy * C:(dy + 1) * C],
                                        xp2[:, 16 * g + dy: 16 * g + dy + 16: 2,
                                            0: 96: 2],
                                        start=False, stop=(dy == 2))
                                # lrelu(x) = 0.6*x + 0.4*|x| (Abs is in
                                # every act table -> no table reloads)
                                ab1 = pa.tile([C, 8 * 48], f32, tag="ab",
                                              name="ab1", bufs=2)
                                nc.scalar.activation(ab1[:], ps1[:], AF.Abs,
                                                     scale=0.4)
                                nc.vector.scalar_tensor_tensor(
                                    y1p[:, 1 + 8 * g: 9 + 8 * g, 1:49],
                                    ps1[:], 0.6, ab1[:],
                                    op0=ALU.mult, op1=ALU.add)

                            # conv2: 48x48 -> 24x24
                            y2p = pa.tile([C, 26, 26], f16)
                            nc.gpsimd.memset(y2p[:], 0.0)
                            for g in range(2):
                                ps2 = paps.tile([C, 12, 24], f32, tag="cv",
                                                name="ps2")
                                for t in range(9):
                                    dy, dx = t // 3, t % 3
                                    nc.tensor.matmul(
                                        ps2[:], w2sb[:, t * C:(t + 1) * C],
                                        y1p[:, 24 * g + dy: 24 * g + dy + 24: 2,
                                            dx: dx + 48: 2],
                                        start=(t == 0), stop=(t == 8))
                                ab2 = pa.tile([C, 12 * 24], f32, tag="ab",
                                              name="ab2", bufs=2)
                                nc.scalar.activation(ab2[:], ps2[:], AF.Abs,
                                                     scale=0.4)
                                nc.vector.scalar_tensor_tensor(
                                    y2p[:, 1 + 12 * g: 13 + 12 * g, 1:25],
                                    ps2[:], 0.6, ab2[:],
                                    op0=ALU.mult, op1=ALU.add)

                            # conv3: 24x24 -> 12x12 (no activation)
                            ps3 = paps.tile([C, 12, 12], f32, tag="cv",
                                            name="ps3")
                            for t in range(9):
                                dy, dx = t // 3, t % 3
                                nc.tensor.matmul(
                                    ps3[:], w3sb[:, t * C:(t + 1) * C],
                                    y2p[:, dy: dy + 24: 2, dx: dx + 24: 2],
                                    start=(t == 0), stop=(t == 8))
                            nc.vector.tensor_copy(y3v[:, 1:13, :], ps3[:])
                            nc.vector.tensor_copy(y3v[:, 0:1, :],
                                                  ps3[:, 0:1, :])
                            nc.vector.tensor_copy(y3v[:, 13:14, :],
                                                  ps3[:, 11:12, :])

                    # --- A2: upsample blocks fused with gate/projections ---
                    with tc.tile_pool(name="pa2", bufs=1) as pa, \
                         tc.tile_pool(name="paps2", bufs=1,
                                      space="PSUM") as paps:
                        # bilinear x8 vertical (tiny DVE ops)
                        nc.vector.tensor_sub(dv[:], y3v[:, 1:14, :],
                                             y3v[:, 0:13, :])
                        for r in range(8):
                            t = (r + 0.5) / 8 - 0.5
                            kr, b = (0, 1 + t) if r < 4 else (1, t)
                            nc.vector.scalar_tensor_tensor(
                                yvp[:, r:96:8, 1:13], dv[:, kr:kr + 12, :],
                                float(b), y3v[:, kr:kr + 12, :],
                                op0=ALU.mult, op1=ALU.add)
                        nc.vector.tensor_copy(yvp[:, :, 0:1], yvp[:, :, 1:2])
                        nc.vector.tensor_copy(yvp[:, :, 13:14],
                                              yvp[:, :, 12:13])
                        yh3 = yh[:].rearrange("c (r n) -> c r n", r=8)
                        nc.vector.tensor_sub(dh[:], yvp[:, :, 1:14],
                                             yvp[:, :, 0:13])

                        def phi_chunk(ci):
                            sl = slice(ci * 512, (ci + 1) * 512)
                            pps = paps.tile([CI, 512], f32,
                                            tag="pj", name="pps")
                            nc.tensor.matmul(pps[:], pwTsb[:], xg16[:, sl],
                                             start=True, stop=True)
                            # alternate evacuation engine so neither ACT nor
                            # DVE backlog stalls the single-bank psum chain
                            if ci % 2 == 0:
                                nc.scalar.activation(phiA[:, sl], pps[:],
                                                     AF.Copy)
                            else:
                                nc.vector.tensor_copy(phiA[:, sl], pps[:])

                        def gt_group(gg):
                            gps = paps.tile([128, 8 * C], f32, tag="gps",
                                            name="gps")
                            for u in range(8):
                                j = gg * 8 + u
                                nc.tensor.matmul(
                                    gps[:, u * C:(u + 1) * C],
                                    xg16[:, j * 128:(j + 1) * 128],
                                    ETsb[:], start=True, stop=True)
                            nc.vector.tensor_copy(
                                G16[:, gg * 8 * C:(gg + 1) * 8 * C], gps[:])

                        # per block: horizontal upsample -> sigmoid -> gate
                        # mul -> phi chunks / G^T groups / early pass-1
                        # tiles, all pipelined per-block so no engine waits
                        # for the whole upsample. Sigmoid-vs-exp act tables:
                        # all sigmoids are queued before the first exp.
                        cdone = gdone = tdone = 0
                        for r in range(8):
                            t = (r + 0.5) / 8 - 0.5
                            kr, b = (0, 1 + t) if r < 4 else (1, t)
                            out3 = yh3[:, r, :].rearrange("c (h j) -> c h j",
                                                          j=12)
                            dhs = pa.tile([C, 96, 12], f16, tag="dhs",
                                          name="dhs", bufs=3)
                            nc.vector.tensor_scalar(
                                dhs[:], dh[:, :, kr:kr + 12],
                                float(b), None, op0=ALU.mult)
                            eng = nc.vector if r < 6 else nc.gpsimd
                            eng.tensor_add(out3, dhs[:],
                                           yvp[:, :, kr:kr + 12])

                            sl = slice(r * CH, (r + 1) * CH)
                            gt = pa.tile([C, CH], f16, tag="gt", name="gt",
                                         bufs=3)
                            nc.scalar.activation(gt[:], yh[:, sl], AF.Sigmoid)
                            nc.vector.tensor_mul(xg16[:, sl], gt[:],
                                                 x16sb[:, sl])
                            if r == 0:
                                # own-chunk early path: dynamic slices wait
                                # on the full yh/xg16 and park in the 4-deep
                                # wait queues while later blocks flow past
                                yhc = pa.tile([C, CH], f16)
                                xc16 = pa.tile([C, CH], f16)
                                with tc.tile_critical():
                                    pid = nc.vector.partition_id()
                                    col0 = pid * CH
                                    nc.vector.tensor_copy(
                                        yhc[:], yh[:, bass.ds(col0, CH)])
                                    nc.vector.tensor_copy(
                                        xc16[:], x16sb[:, bass.ds(col0, CH)])
                                gtc = pa.tile([C, CH], f16)
                                nc.scalar.activation(gtc[:], yhc[:],
                                                     AF.Sigmoid)
                                nc.vector.tensor_mul(xgc[:], gtc[:], xc16[:])
                                for si, (o0, w) in enumerate(SUBS):
                                    tps = paps.tile([CI, 512], f32,
                                                    tag="pj", name="tps")
                                    nc.tensor.matmul(tps[:, 0:w], twTsb[:],
                                                     xgc[:, o0:o0 + w],
                                                     start=True, stop=True)
                                    nc.vector.tensor_copy(thc[:, o0:o0 + w],
                                                          tps[:, 0:w])
                            while (cdone + 1) * 512 <= (r + 1) * CH:
                                phi_chunk(cdone)
                                cdone += 1
                                if (gdone + 1) * 1024 <= (r + 1) * CH:
                                    gt_group(gdone)
                                    gdone += 1
                                # early pass-1 tiles once theta is ready
                                # (~chunk 11): only DVE-schraudolph tiles may
                                # run before the last sigmoid is queued (ACT
                                # exp would force an act-table reload)
                                while cdone >= 11 and tdone < len(EARLY_D) \
                                        and EARLY_D[tdone] <= \
                                        4 * (cdone - 1):
                                    pass1_tile(EARLY_D[tdone], "D")
                                    tdone += 1
                        while gdone < 9:
                            gt_group(gdone)
                            gdone += 1
                        while tdone < len(EARLY_D):
                            pass1_tile(EARLY_D[tdone], "D")
                            tdone += 1

                if debug:
                    dbg_th = nc.dram_tensor("dbg_th", [CI, CH], f16,
                                            kind="ExternalOutput")
                    dbg_s0 = nc.dram_tensor("dbg_s0", [128, 2 * CH], f16,
                                            kind="ExternalOutput")
                    nc.sync.dma_start(dbg_th[:], thc[:])
                    nc.sync.dma_start(dbg_s0[:, 0:CH], s_sl(0))
                    nc.sync.dma_start(dbg_s0[:, CH:2 * CH], s_sl(1))

                # ============ PASS 1 + interleaved PASS 2 chunks ============
                with tc.tile_pool(name="scache", bufs=1) as scp:
                    s_cacheB = scp.tile([128, (MT - NA) * CH], f16)

                    def allreduce(ci, j0, j1):
                        nc.sync.dma_start(zin[ci][:], zsum[:, j0:j1])
                        if single:
                            nc.sync.dma_start(zout[ci][:], zin[ci][:])
                        else:
                            nc.gpsimd.collective_compute(
                                "AllReduce", ALU.add,
                                replica_groups=[list(range(NCORES))],
                                ins=[zin[ci].opt()], outs=[zout[ci].opt()])

                    with tc.tile_pool(name="p2", bufs=1) as p2, \
                         tc.tile_pool(name="p2ps", bufs=2,
                                      space="PSUM") as p2ps:

                        def scale_G(ci, j0, j1):
                            w = j1 - j0
                            zf = p2.tile([128, ARC[0]], f32, tag="zf",
                                         name="zf", bufs=2)
                            nc.sync.dma_start(zf[:, 0:w], zout[ci][:])
                            rz = p2.tile([128, ARC[0]], f16, tag="rz",
                                         name="rz", bufs=2)
                            with nc.allow_low_precision(
                                    reason="1/Z weights tolerate fp16"):
                                nc.vector.reciprocal(rz[:, 0:w],
                                                     zf[:, 0:w])
                            # split the scale so each pass-2 half-chain
                            # gates on only its half of G (and the slow Pool
                            # op is half as long); last chunk on DVE (tail)
                            eng = nc.vector if j1 == MT else nc.gpsimd
                            h = w // 2
                            for a0, a1 in ((0, h), (h, w)):
                                if a0 == a1:
                                    continue
                                rzb = rz[:, a0:a1].unsqueeze(-1).to_broadcast(
                                    (128, a1 - a0, C))
                                eng.tensor_mul(G3[:, j0 + a0:j0 + a1, :],
                                               G3[:, j0 + a0:j0 + a1, :],
                                               rzb)

                        def pass2_ops(ci, j0, j1, first, last):
                            # two interleaved half-chains per sub (both psum
                            # bufs) hide the serial accumulation latency
                            nt = j1 - j0
                            h0 = nt // 2
                            for o0, w in SUBS:
                                ypsA = p2ps.tile([64, 512], f32, tag="yps",
                                                 name="ypsA")
                                ypsB = p2ps.tile([64, 512], f32, tag="yps",
                                                 name="ypsB")
                                def mk(yps, jj, u, lu, o0=o0, w=w):
                                    return lambda: nc.tensor.matmul(
                                        yps[:, 0:w], G3[:, jj, :],
                                        s_sl(jj)[:, o0:o0 + w],
                                        start=(u == 0), stop=(u == lu),
                                        skip_group_check=True)
                                for u in range(h0):
                                    yield mk(ypsA, j0 + u, u, h0 - 1)
                                    yield mk(ypsB, j0 + h0 + u, u,
                                             nt - h0 - 1)
                                if nt - h0 > h0:
                                    yield mk(ypsB, j1 - 1, nt - h0 - 1,
                                             nt - h0 - 1)
                                if first:
                                    yield lambda yps=ypsA, o0=o0, w=w: \
                                        nc.vector.tensor_add(
                                            outsb[:, o0:o0 + w],
                                            yps[:, 0:w], xgc[:, o0:o0 + w])
                                else:
                                    yield lambda yps=ypsA, o0=o0, w=w: \
                                        nc.vector.tensor_add(
                                            outsb[:, o0:o0 + w],
                                            outsb[:, o0:o0 + w],
                                            yps[:, 0:w])
                                if last:
                                    def fin(yps=ypsB, o0=o0, w=w):
                                        nc.vector.tensor_add(
                                            outsb[:, o0:o0 + w],
                                            outsb[:, o0:o0 + w],
                                            yps[:, 0:w])
                                        nc.sync.dma_start(
                                            out_io[:, o0:o0 + w],
                                            outsb[:, o0:o0 + w])
                                    yield fin
                                else:
                                    yield lambda yps=ypsB, o0=o0, w=w: \
                                        nc.vector.tensor_add(
                                            outsb[:, o0:o0 + w],
                                            outsb[:, o0:o0 + w],
                                            yps[:, 0:w])

                        # chunked AR/pass-2 pipeline: AR(ci) fires right
                        # after its last tile; its pass-2 starts 3 tiles
                        # later (AR latency cover) and drains at ~5 ops per
                        # pass-1 tile so PE never starves the exp engines
                        NC_ = len(ARC)
                        B = [0]
                        for w in ARC:
                            B.append(B[-1] + w)
                        rest = [j for j in range(MT) if j not in EARLY_D]
                        # AR ci can fire once all tiles < B[ci+1] are done
                        arpt = {}
                        seen = set(EARLY_D)
                        for i, j in enumerate(rest):
                            seen.add(j)
                            for ci in range(NC_):
                                if B[ci + 1] - 1 in seen and \
                                        all(x in seen
                                            for x in range(B[ci + 1])):
                                    if ci not in arpt.values():
                                        arpt[i] = ci
                        scale_at = {}
                        gen_at = {}
                        for i, ci in arpt.items():
                            scale_at.setdefault(i + 1, ci)
                            gen_at.setdefault(min(i + 9, len(rest) - 3), ci)
                        active = []
                        for i, j in enumerate(rest):
                            kls = "D" if i % 7 in (2, 5) else "A"
                            pass1_tile(j, kls)
                            if i in arpt:
                                ci = arpt[i]
                                allreduce(ci, B[ci], B[ci + 1])
                            if i in scale_at and scale_at[i] != NC_ - 1:
                                ci = scale_at[i]
                                scale_G(ci, B[ci], B[ci + 1])
                            if i in gen_at and gen_at[i] != NC_ - 1:
                                ci = gen_at[i]
                                active.append(pass2_ops(
                                    ci, B[ci], B[ci + 1],
                                    first=(ci == 0), last=False))
                            done = 0
                            while active and done < PACE:
                                op = next(active[0], None)
                                if op is None:
                                    active.pop(0)
                                else:
                                    op()
                                    done += 1
                        for gen in active:
                            for op in gen:
                                op()
                        ci = NC_ - 1
                        scale_G(ci, B[ci], B[ci + 1])
                        for op in pass2_ops(ci, B[ci], B[ci + 1],
                                            first=False, last=True):
                            op()

    nc.compile()
    return nc


def get_program():
    if "nc" not in _compiled:
        _compiled["nc"] = _build()
    return _compiled["nc"]


def make_in_maps(inputs):
    f16 = np.float16
    x = np.asarray(inputs["x"], np.float32).reshape(C, H, W)
    xpad = np.zeros((C, 98, 98), f16)
    xpad[:, 1:97, 1:97] = x.astype(f16)
    # permuted layout: n' = (w%8)*1152 + h*12 + w//8
    x16p = np.ascontiguousarray(
        x.reshape(C, H, W // 8, 8).transpose(0, 3, 1, 2).reshape(C, N)
    ).astype(f16)

    def conv_w(w):
        return np.ascontiguousarray(
            np.asarray(w, np.float32).transpose(1, 2, 3, 0).reshape(C, 9 * C)
        ).astype(f16)

    # xpadB = xpad shifted one column left (for the 2-tap paired conv1)
    xpadB = np.zeros((C, 98, 98), f16)
    xpadB[:, :, 0:97] = xpad[:, :, 1:98]
    # paired conv1 weights: w1p rows 0-63 tap (dy,0), rows 64-127 tap (dy,1)
    w1f = np.asarray(inputs["d1_w"], np.float32).transpose(1, 2, 3, 0)
    w1p = np.concatenate([
        np.concatenate([w1f[:, dy, 0, :], w1f[:, dy, 1, :]], axis=0)
        for dy in range(3)], axis=1).astype(f16)
    w1s = np.concatenate([w1f[:, dy, 2, :] for dy in range(3)],
                         axis=1).astype(f16)

    gw = np.asarray(inputs["g_w"], np.float32)[:, :, 0, 0]
    Ww = np.asarray(inputs["W_w"], np.float32)[:, :, 0, 0]
    base = {
        "xpad": xpad,
        "xpadB": xpadB,
        "w1p": np.ascontiguousarray(w1p),
        "w1s": np.ascontiguousarray(w1s),
        "x16": x16p,
        "w1": conv_w(inputs["d1_w"]),
        "w2": conv_w(inputs["d2_w"]),
        "w3": conv_w(inputs["d3_w"]),
        "twT": np.ascontiguousarray(
            np.asarray(inputs["th_w"], np.float32)[:, :, 0, 0].T).astype(f16),
        "pwTA": np.ascontiguousarray(
            np.asarray(inputs["ph_w"], np.float32)[:, :, 0, 0].T
            * A_EXP).astype(f16),
        "ET": np.ascontiguousarray((Ww @ gw).T).astype(f16),
    }
    return [dict(base) for _ in range(NCORES)]


def kernel(**inputs):
    from concourse import bass_utils

    nc = get_program()
    in_maps = make_in_maps(inputs)
    res = bass_utils.run_bass_kernel_spmd(nc, in_maps,
                                          core_ids=list(range(NCORES)))
    # gather permuted chunks -> full permuted [C, N] -> un-permute
    outp = np.concatenate([res.results[k]["out"] for k in range(NCORES)],
                          axis=1)
    out = outp.reshape(C, 8, H, W // 8).transpose(0, 2, 3, 1).reshape(C, H, W)
    return out.reshape(1, C, H, W).astype(np.float32)


# revision 9
# speedup vs baseline: 1.0383x; 1.0131x over previous
"""Trainium2 Bass kernel for AttentiveNonLocalBlock2D (v3).

Per-core SPMD over 8 NeuronCores, sequence-parallel over N=H*W with a
global pixel permutation n' = (w%8)*1152 + h*12 + w//8 so the x8 bilinear
upsample writes contiguous blocks. Core k owns permuted block k (w%8==k).

  Phase A: 3x stride-2 conv gating unit (fp16 PE) -> bilinear x8 upsample
    (fp16 2x STTs) -> per-block sigmoid (ACT) * x (DVE fp16 2x) -> xg16;
    phi cache prescaled by A=2^10/ln2; theta (own chunk); G^T fp16.
  Pass 1 per m-tile: psum = A*f = A * phi^T theta (fp16 PE), then one of
    A: ACT exp(psum/A - 5) -> fp16 s-cache, Z via accum_out;
    D: DVE Schraudolph max(psum+S1,0)->int16 (bitcast fp16), DVE Z-reduce;
    P: ACT Copy psum->fp16, gpsimd Schraudolph in-place, DVE Z-reduce.
    (per-m-tile exp bias differences cancel in the n-softmax)
  Z AllReduced in 3 chunks (36/24/12 m-tiles); pass-2 chunks interleaved
  under pass 1; short tail = last AR + 12-tile pass 2.
  Pass 2: out[64, n] = sum_m (G[:,m]/Z[m]) s[m,n], one psum chain per
    512-col sub per chunk, + residual xg.
"""

import sys

if "/opt/trn_rl_repo" not in sys.path:
    sys.path.insert(0, "/opt/trn_rl_repo")

import numpy as np
import os

NCORES = 8
C, CI, H, W = 64, 32, 96, 96
N = H * W            # 9216
CH = N // NCORES     # 1152 pixels per core (one w%8 phase)
MT = N // 128        # 72 m-tiles
SUBS = ((0, 512), (512, 512), (1024, 128))
DMOD = int(os.environ.get("DMOD", "3"))
PACE = int(os.environ.get("PACE", "3"))
GOFF = int(os.environ.get("GOFF", "12"))
NA = 24              # m-tiles in the early s-cache pool
ARC = tuple(int(x) for x in
            os.environ.get("ARC", "24,16,12,12,8").split(","))

A_EXP = float(2.0**10 / np.log(2.0))   # fp16 Schraudolph scale
B_SCH = 5.94                            # exp bias for schraudolph tiles
C_SCH = -189.0                          # truncation-centering correction
S1_SCH = float(15360.0 + C_SCH - A_EXP * B_SCH)
EXP_BIAS = -5.0                         # bias for ACT tiles


# 8 early DVE-schraudolph tiles run during phase A2 (no ACT exp allowed
# before the last sigmoid); later tiles are classed by emission order
EARLY_D = (1, 4, 7, 10, 13, 16, 19, 22)


_compiled = {}


def _build(single=False, debug=False):
    import concourse.bacc as bacc
    import concourse.bass as bass
    import concourse.mybir as mybir
    import concourse.tile as tile

    f16 = mybir.dt.float16
    f32 = mybir.dt.float32
    i16 = mybir.dt.int16
    AF = mybir.ActivationFunctionType
    ALU = mybir.AluOpType
    X = mybir.AxisListType.X

    nc = bacc.Bacc("TRN2", target_bir_lowering=False, debug=False,
                   num_devices=1 if single else NCORES)

    xpad_io = nc.dram_tensor("xpad", [C, 98, 98], f16, kind="ExternalInput")
    xpadB_io = nc.dram_tensor("xpadB", [C, 98, 98], f16, kind="ExternalInput")
    wpk_io = nc.dram_tensor("wpk", [2 * C, 1664], f16, kind="ExternalInput")
    x16_io = nc.dram_tensor("x16", [C, N], f16, kind="ExternalInput")
    w1_io = nc.dram_tensor("w1", [C, 9 * C], f16, kind="ExternalInput")
    w2_io = nc.dram_tensor("w2", [C, 9 * C], f16, kind="ExternalInput")
    w3_io = nc.dram_tensor("w3", [C, 9 * C], f16, kind="ExternalInput")
    twT_io = nc.dram_tensor("twT", [C, CI], f16, kind="ExternalInput")
    pwTA_io = nc.dram_tensor("pwTA", [C, CI], f16, kind="ExternalInput")
    ET_io = nc.dram_tensor("ET", [C, C], f16, kind="ExternalInput")
    out_io = nc.dram_tensor("out", [C, CH], f32, kind="ExternalOutput")

    with tile.TileContext(nc) as tc:
        with tc.tile_pool(name="persist", bufs=1) as pp, \
             tc.tile_pool(name="dram", bufs=1, space="DRAM") as dp:
            zsum = pp.tile([128, MT], f32)
            nb5 = pp.tile([128, 1], f32)
            nc.gpsimd.memset(nb5[:], EXP_BIAS)
            zin = []
            zout = []
            for ci, w in enumerate(ARC):
                zin.append(dp.tile([128, w], f32, name=f"zin{ci}"))
                zout.append(dp.tile([128, w], f32, addr_space="Shared",
                                    name=f"zout{ci}"))

            with tc.tile_pool(name="hand", bufs=1) as hp, \
                 tc.tile_pool(name="scA", bufs=1) as scpA, \
                 tc.tile_pool(name="p1ps", bufs=2, space="PSUM") as p1ps:
                phiA = hp.tile([CI, N], f16)          # A * phi
                thc = hp.tile([CI, CH], f16)
                G16 = hp.tile([128, MT * C], f16)
                G3 = G16[:].rearrange("p (j c) -> p j c", c=C)
                xgc = hp.tile([C, CH], f16)           # own-chunk x_gated
                outsb = hp.tile([C, CH], f32)
                s_cacheA = scpA.tile([128, NA * CH], f16)

                def s_sl(j):
                    if j < NA:
                        return s_cacheA[:, j * CH:(j + 1) * CH]
                    r = MT - 1 - j
                    return s_cacheB[:, r * CH:(r + 1) * CH]

                def pass1_tile(j, k):
                    fps = p1ps.tile([128, CH], f32, tag="fps", name="fps")
                    for o0, w in SUBS:
                        nc.tensor.matmul(fps[:, o0:o0 + w],
                                         phiA[:, j * 128:(j + 1) * 128],
                                         thc[:, o0:o0 + w],
                                         start=True, stop=True)
                    ssl = s_sl(j)
                    if k == "A":
                        nc.scalar.activation(ssl, fps[:], AF.Exp,
                                             bias=nb5[:],
                                             scale=float(1.0 / A_EXP),
                                             accum_out=zsum[:, j:j + 1])
                    else:
                        nc.vector.tensor_scalar(ssl.bitcast(i16), fps[:],
                                                S1_SCH, 0.0,
                                                op0=ALU.add, op1=ALU.max)
                        # Z row-sum via 4x-mode identity pass with accum
                        nc.vector.tensor_scalar(ssl, ssl, 0.0, None,
                                                op0=ALU.add, op1=ALU.add,
                                                accum_out=zsum[:, j:j + 1])

                # ==================== PHASE A ====================
                with tc.tile_pool(name="mid", bufs=1) as pm, \
                     tc.tile_pool(name="pup", bufs=1) as pu:
                    yh = pm.tile([C, N], f16)     # permuted upsampled logits
                    xg16 = pm.tile([C, N], f16)   # permuted x_gated
                    y3v = pu.tile([C, 14, 12], f32)
                    yvp = pu.tile([C, 96, 14], f16)
                    dv = pu.tile([C, 13, 12], f32)
                    dh = pu.tile([C, 96, 13], f16)

                    # all weights arrive in one packed DMA (vs 7 queue
                    # round-trips ahead of the xpad chunks)
                    wpk = pm.tile([2 * C, 1664], f16)
                    # conv1 weights first so group 0 starts immediately;
                    # the rest follows in a second transfer
                    nc.sync.dma_start(wpk[:, 0:384], wpk_io[:, 0:384])
                    w1psb = wpk[:, 0:192]
                    w1ssb = wpk[0:C, 192:384]
                    w2sb = wpk[0:C, 384:960]
                    w3sb = wpk[0:C, 960:1536]
                    twTsb = wpk[0:C, 1536:1568]
                    pwTsb = wpk[0:C, 1568:1600]
                    ETsb = wpk[0:C, 1600:1664]

                    # --- A1: convs (pool closes before A2 to free SBUF) ---
                    with tc.tile_pool(name="pcv", bufs=1) as pa:
                        # xp2: partitions 0-63 = xpad, 64-127 = xpad shifted
                        # one column left, so one matmul covers 2 taps; the
                        # shifted copy loads via the gpsimd software DGE in
                        # parallel with the sync HWDGE stream
                        xp2 = pa.tile([2 * C, 98, 98], f16)
                        for b in range(4):
                            r0, r1 = 26 * b, min(26 * b + 26, 98)
                            nc.sync.dma_start(xp2[0:C, r0:r1, :],
                                              xpad_io[:, r0:r1, :])
                            nc.gpsimd.dma_start(xp2[C:2 * C, r0:r1, :],
                                                xpadB_io[:, r0:r1, :])
                        # remaining weights after the conv-critical xpad
                        nc.sync.dma_start(wpk[:, 384:1664],
                                          wpk_io[:, 384:1664])
                        x16sb = pm.tile([C, N], f16)
                        nc.sync.dma_start(x16sb[:, 0:4608], x16_io[:, 0:4608])
                        nc.sync.dma_start(x16sb[:, 4608:N], x16_io[:, 4608:N])

                        with tc.tile_pool(name="cvps", bufs=2,
                                          space="PSUM") as paps:
                            # conv1: 96x96 -> 48x48, stride 2, pad 1, lrelu
                            y1p = pa.tile([C, 50, 50], f16)
                            nc.gpsimd.memset(y1p[:], 0.0)
                            for g in range(6):
                                ps1 = paps.tile([C, 8, 48], f32, tag="cv",
                                                name="ps1")
                                # singles (xpad only) first: they don't wait
                                # for the software-DGE xpadB stream
                                for dy in range(3):
                                    nc.tensor.matmul(
                                        ps1[:], w1ssb[:, dy * C:(dy + 1) * C],
                                        xp2[0:C,
                                            16 * g + dy: 16 * g + dy + 16: 2,
                                            2: 98: 2],
                                        start=(dy == 0), stop=False)
                                for dy in range(3):
                                    nc.tensor.matmul(
                                        ps1[:], w1psb[:, dy * C:(dy + 1) * C],
                                        xp2[:, 16 * g + dy: 16 * g + dy + 16: 2,
                                            0: 96: 2],
                                        start=False, stop=(dy == 2))
                                # lrelu(x) = 0.6*x + 0.4*|x| (Abs is in
                                # every act table -> no table reloads)
                                ab1 = pa.tile([C, 8 * 48], f32, tag="ab",
                                              name="ab1", bufs=2)
                                nc.scalar.activation(ab1[:], ps1[:], AF.Abs,
                                                     scale=0.4)
                                nc.vector.scalar_tensor_tensor(
                                    y1p[:, 1 + 8 * g: 9 + 8 * g, 1:49],
                                    ps1[:], 0.6, ab1[:],
                                    op0=ALU.mult, op1=ALU.add)

                            # conv2: 48x48 -> 24x24
                            y2p = pa.tile([C, 26, 26], f16)
                            nc.gpsimd.memset(y2p[:], 0.0)
                            for g in range(2):
                                ps2 = paps.tile([C, 12, 24], f32, tag="cv",
                                                name="ps2")
                                for t in range(9):
                                    dy, dx = t // 3, t % 3
                                    nc.tensor.matmul(
                                        ps2[:], w2sb[:, t * C:(t + 1) * C],
                                        y1p[:, 24 * g + dy: 24 * g + dy + 24: 2,
                                            dx: dx + 48: 2],
                                        start=(t == 0), stop=(t == 8))
                                ab2 = pa.tile([C, 12 * 24], f32, tag="ab",
                                              name="ab2", bufs=2)
                                nc.scalar.activation(ab2[:], ps2[:], AF.Abs,
                                                     scale=0.4)
                                nc.vector.scalar_tensor_tensor(
                                    y2p[:, 1 + 12 * g: 13 + 12 * g, 1:25],
                                    ps2[:], 0.6, ab2[:],
                                    op0=ALU.mult, op1=ALU.add)

                            # conv3: 24x24 -> 12x12 (no activation)
                            ps3 = paps.tile([C, 12, 12], f32, tag="cv",
                                            name="ps3")
                            for t in range(9):
                                dy, dx = t // 3, t % 3
                                nc.tensor.matmul(
                                    ps3[:], w3sb[:, t * C:(t + 1) * C],
                                    y2p[:, dy: dy + 24: 2, dx: dx + 24: 2],
                                    start=(t == 0), stop=(t == 8))
                            nc.vector.tensor_copy(y3v[:, 1:13, :], ps3[:])
                            nc.vector.tensor_copy(y3v[:, 0:1, :],
                                                  ps3[:, 0:1, :])
                            nc.vector.tensor_copy(y3v[:, 13:14, :],
                                                  ps3[:, 11:12, :])

                    # --- A2: upsample blocks fused with gate/projections ---
                    with tc.tile_pool(name="pa2", bufs=1) as pa, \
                         tc.tile_pool(name="paps2", bufs=1,
                                      space="PSUM") as paps:
                        # bilinear x8 vertical (tiny DVE ops)
                        nc.vector.tensor_sub(dv[:], y3v[:, 1:14, :],
                                             y3v[:, 0:13, :])
                        for r in range(8):
                            t = (r + 0.5) / 8 - 0.5
                            kr, b = (0, 1 + t) if r < 4 else (1, t)
                            nc.vector.scalar_tensor_tensor(
                                yvp[:, r:96:8, 1:13], dv[:, kr:kr + 12, :],
                                float(b), y3v[:, kr:kr + 12, :],
                                op0=ALU.mult, op1=ALU.add)
                        nc.vector.tensor_copy(yvp[:, :, 0:1], yvp[:, :, 1:2])
                        nc.vector.tensor_copy(yvp[:, :, 13:14],
                                              yvp[:, :, 12:13])
                        yh3 = yh[:].rearrange("c (r n) -> c r n", r=8)
                        nc.vector.tensor_sub(dh[:], yvp[:, :, 1:14],
                                             yvp[:, :, 0:13])

                        def phi_chunk(ci):
                            sl = slice(ci * 512, (ci + 1) * 512)
                            pps = paps.tile([CI, 512], f32,
                                            tag="pj", name="pps")
                            nc.tensor.matmul(pps[:], pwTsb, xg16[:, sl],
                                             start=True, stop=True)
                            # alternate evacuation engine so neither ACT nor
                            # DVE backlog stalls the single-bank psum chain
                            if ci % 2 == 0:
                                nc.scalar.activation(phiA[:, sl], pps[:],
                                                     AF.Copy)
                            else:
                                nc.vector.tensor_copy(phiA[:, sl], pps[:])

                        def gt_group(gg):
                            gps = paps.tile([128, 8 * C], f32, tag="gps",
                                            name="gps")
                            for u in range(8):
                                j = gg * 8 + u
                                nc.tensor.matmul(
                                    gps[:, u * C:(u + 1) * C],
                                    xg16[:, j * 128:(j + 1) * 128],
                                    ETsb, start=True, stop=True)
                            nc.vector.tensor_copy(
                                G16[:, gg * 8 * C:(gg + 1) * 8 * C], gps[:])

                        # per block: horizontal upsample -> sigmoid -> gate
                        # mul -> phi chunks / G^T groups / early pass-1
                        # tiles, all pipelined per-block so no engine waits
                        # for the whole upsample. Sigmoid-vs-exp act tables:
                        # all sigmoids are queued before the first exp.
                        cdone = gdone = tdone = 0
                        for r in range(8):
                            t = (r + 0.5) / 8 - 0.5
                            kr, b = (0, 1 + t) if r < 4 else (1, t)
                            out3 = yh3[:, r, :].rearrange("c (h j) -> c h j",
                                                          j=12)
                            dhs = pa.tile([C, 96, 12], f16, tag="dhs",
                                          name="dhs", bufs=3)
                            nc.vector.tensor_scalar(
                                dhs[:], dh[:, :, kr:kr + 12],
                                float(b), None, op0=ALU.mult)
                            eng = nc.vector if r < 6 else nc.gpsimd
                            eng.tensor_add(out3, dhs[:],
                                           yvp[:, :, kr:kr + 12])

                            sl = slice(r * CH, (r + 1) * CH)
                            gt = pa.tile([C, CH], f16, tag="gt", name="gt",
                                         bufs=3)
                            nc.scalar.activation(gt[:], yh[:, sl], AF.Sigmoid)
                            nc.vector.tensor_mul(xg16[:, sl], gt[:],
                                                 x16sb[:, sl])
                            if r == 0:
                                # own-chunk early path: dynamic slices wait
                                # on the full yh/xg16 and park in the 4-deep
                                # wait queues while later blocks flow past
                                yhc = pa.tile([C, CH], f16)
                                xc16 = pa.tile([C, CH], f16)
                                with tc.tile_critical():
                                    pid = nc.vector.partition_id()
                                    col0 = pid * CH
                                    nc.vector.tensor_copy(
                                        yhc[:], yh[:, bass.ds(col0, CH)])
                                    nc.vector.tensor_copy(
                                        xc16[:], x16sb[:, bass.ds(col0, CH)])
                                gtc = pa.tile([C, CH], f16)
                                nc.scalar.activation(gtc[:], yhc[:],
                                                     AF.Sigmoid)
                                nc.vector.tensor_mul(xgc[:], gtc[:], xc16[:])
                                for si, (o0, w) in enumerate(SUBS):
                                    tps = paps.tile([CI, 512], f32,
                                                    tag="pj", name="tps")
                                    nc.tensor.matmul(tps[:, 0:w], twTsb,
                                                     xgc[:, o0:o0 + w],
                                                     start=True, stop=True)
                                    nc.vector.tensor_copy(thc[:, o0:o0 + w],
                                                          tps[:, 0:w])
                            while (cdone + 1) * 512 <= (r + 1) * CH:
                                phi_chunk(cdone)
                                cdone += 1
                                if (gdone + 1) * 1024 <= (r + 1) * CH:
                                    gt_group(gdone)
                                    gdone += 1
                                # early pass-1 tiles once theta is ready
                                # (~chunk 11): only DVE-schraudolph tiles may
                                # run before the last sigmoid is queued (ACT
                                # exp would force an act-table reload)
                                while cdone >= 11 and tdone < len(EARLY_D) \
                                        and EARLY_D[tdone] <= \
                                        4 * (cdone - 1):
                                    pass1_tile(EARLY_D[tdone], "D")
                                    tdone += 1
                        while gdone < 9:
                            gt_group(gdone)
                            gdone += 1
                        while tdone < len(EARLY_D):
                            pass1_tile(EARLY_D[tdone], "D")
                            tdone += 1

                if debug:
                    dbg_th = nc.dram_tensor("dbg_th", [CI, CH], f16,
                                            kind="ExternalOutput")
                    dbg_s0 = nc.dram_tensor("dbg_s0", [128, 2 * CH], f16,
                                            kind="ExternalOutput")
                    nc.sync.dma_start(dbg_th[:], thc[:])
                    nc.sync.dma_start(dbg_s0[:, 0:CH], s_sl(0))
                    nc.sync.dma_start(dbg_s0[:, CH:2 * CH], s_sl(1))

                # ============ PASS 1 + interleaved PASS 2 chunks ============
                with tc.tile_pool(name="scache", bufs=1) as scp:
                    s_cacheB = scp.tile([128, (MT - NA) * CH], f16)

                    def allreduce(ci, j0, j1):
                        nc.sync.dma_start(zin[ci][:], zsum[:, j0:j1])
                        if single:
                            nc.sync.dma_start(zout[ci][:], zin[ci][:])
                        else:
                            nc.gpsimd.collective_compute(
                                "AllReduce", ALU.add,
                                replica_groups=[list(range(NCORES))],
                                ins=[zin[ci].opt()], outs=[zout[ci].opt()])

                    with tc.tile_pool(name="p2", bufs=1) as p2, \
                         tc.tile_pool(name="p2ps", bufs=2,
                                      space="PSUM") as p2ps:

                        def scale_G(ci, j0, j1):
                            w = j1 - j0
                            zf = p2.tile([128, ARC[0]], f32, tag="zf",
                                         name="zf", bufs=2)
                            nc.sync.dma_start(zf[:, 0:w], zout[ci][:])
                            rz = p2.tile([128, ARC[0]], f16, tag="rz",
                                         name="rz", bufs=2)
                            with nc.allow_low_precision(
                                    reason="1/Z weights tolerate fp16"):
                                nc.vector.reciprocal(rz[:, 0:w],
                                                     zf[:, 0:w])
                            # split the scale so each pass-2 half-chain
                            # gates on only its half of G (and the slow Pool
                            # op is half as long); last chunk on DVE (tail)
                            eng = nc.vector if j1 == MT else nc.gpsimd
                            h = w // 2
                            for a0, a1 in ((0, h), (h, w)):
                                if a0 == a1:
                                    continue
                                rzb = rz[:, a0:a1].unsqueeze(-1).to_broadcast(
                                    (128, a1 - a0, C))
                                eng.tensor_mul(G3[:, j0 + a0:j0 + a1, :],
                                               G3[:, j0 + a0:j0 + a1, :],
                                               rzb)

                        def pass2_ops(ci, j0, j1, first, last):
                            # two interleaved half-chains per sub (both psum
                            # bufs) hide the serial accumulation latency
                            nt = j1 - j0
                            h0 = nt // 2
                            for o0, w in SUBS:
                                ypsA = p2ps.tile([64, 512], f32, tag="yps",
                                                 name="ypsA")
                                ypsB = p2ps.tile([64, 512], f32, tag="yps",
                                                 name="ypsB")
                                def mk(yps, jj, u, lu, o0=o0, w=w):
                                    return lambda: nc.tensor.matmul(
                                        yps[:, 0:w], G3[:, jj, :],
                                        s_sl(jj)[:, o0:o0 + w],
                                        start=(u == 0), stop=(u == lu),
                                        skip_group_check=True)
                                for u in range(h0):
                                    yield mk(ypsA, j0 + u, u, h0 - 1)
                                    yield mk(ypsB, j0 + h0 + u, u,
                                             nt - h0 - 1)
                                if nt - h0 > h0:
                                    yield mk(ypsB, j1 - 1, nt - h0 - 1,
                                             nt - h0 - 1)
                                if first:
                                    yield lambda yps=ypsA, o0=o0, w=w: \
                                        nc.vector.tensor_add(
                                            outsb[:, o0:o0 + w],
                                            yps[:, 0:w], xgc[:, o0:o0 + w])
                                else:
                                    yield lambda yps=ypsA, o0=o0, w=w: \
                                        nc.vector.tensor_add(
                                            outsb[:, o0:o0 + w],
                                            outsb[:, o0:o0 + w],
                                            yps[:, 0:w])
                                if last:
                                    def fin(yps=ypsB, o0=o0, w=w):
                                        nc.vector.tensor_add(
                                            outsb[:, o0:o0 + w],
                                            outsb[:, o0:o0 + w],
                                            yps[:, 0:w])
                                        nc.sync.dma_start(
                                            out_io[:, o0:o0 + w],
                                            outsb[:, o0:o0 + w])
                                    yield fin
                                else:
                                    yield lambda yps=ypsB, o0=o0, w=w: \
                                        nc.vector.tensor_add(
                                            outsb[:, o0:o0 + w],
                                            outsb[:, o0:o0 + w],
                                            yps[:, 0:w])

                        # chunked AR/pass-2 pipeline: AR(ci) fires right
                        # after its last tile; its pass-2 starts 3 tiles
                        # later (AR latency cover) and drains at ~5 ops per
                        # pass-1 tile so PE never starves the exp engines
                        NC_ = len(ARC)
                        B = [0]
                        for w in ARC:
                            B.append(B[-1] + w)
                        rest = [j for j in range(MT) if j not in EARLY_D]
                        # AR ci can fire once all tiles < B[ci+1] are done
                        arpt = {}
                        seen = set(EARLY_D)
                        for i, j in enumerate(rest):
                            seen.add(j)
                            for ci in range(NC_):
                                if B[ci + 1] - 1 in seen and \
                                        all(x in seen
                                            for x in range(B[ci + 1])):
                                    if ci not in arpt.values():
                                        arpt[i] = ci
                        scale_at = {}
                        gen_at = {}
                        for i, ci in arpt.items():
                            scale_at.setdefault(i + 1, ci)
                            gen_at.setdefault(min(i + GOFF, len(rest) - 3), ci)
                        active = []
                        for i, j in enumerate(rest):
                            kls = "D" if i % 7 in (2, 5) else "A"
                            pass1_tile(j, kls)
                            if i in arpt:
                                ci = arpt[i]
                                allreduce(ci, B[ci], B[ci + 1])
                            if i in scale_at and scale_at[i] != NC_ - 1:
                                ci = scale_at[i]
                                scale_G(ci, B[ci], B[ci + 1])
                            if i in gen_at and gen_at[i] != NC_ - 1:
                                ci = gen_at[i]
                                active.append(pass2_ops(
                                    ci, B[ci], B[ci + 1],
                                    first=(ci == 0), last=False))
                            done = 0
                            while active and done < PACE:
                                op = next(active[0], None)
                                if op is None:
                                    active.pop(0)
                                else:
                                    op()
                                    done += 1
                        for gen in active:
                            for op in gen:
                                op()
                        ci = NC_ - 1
                        scale_G(ci, B[ci], B[ci + 1])
                        for op in pass2_ops(ci, B[ci], B[ci + 1],
                                            first=False, last=True):
                            op()

    nc.compile()
    return nc


def get_program():
    if "nc" not in _compiled:
        _compiled["nc"] = _build()
    return _compiled["nc"]


def make_in_maps(inputs):
    f16 = np.float16
    x = np.asarray(inputs["x"], np.float32).reshape(C, H, W)
    xpad = np.zeros((C, 98, 98), f16)
    xpad[:, 1:97, 1:97] = x.astype(f16)
    # permuted layout: n' = (w%8)*1152 + h*12 + w//8
    x16p = np.ascontiguousarray(
        x.reshape(C, H, W // 8, 8).transpose(0, 3, 1, 2).reshape(C, N)
    ).astype(f16)

    def conv_w(w):
        return np.ascontiguousarray(
            np.asarray(w, np.float32).transpose(1, 2, 3, 0).reshape(C, 9 * C)
        ).astype(f16)

    # xpadB = xpad shifted one column left (for the 2-tap paired conv1)
    xpadB = np.zeros((C, 98, 98), f16)
    xpadB[:, :, 0:97] = xpad[:, :, 1:98]
    # paired conv1 weights: w1p rows 0-63 tap (dy,0), rows 64-127 tap (dy,1)
    w1f = np.asarray(inputs["d1_w"], np.float32).transpose(1, 2, 3, 0)
    w1p = np.concatenate([
        np.concatenate([w1f[:, dy, 0, :], w1f[:, dy, 1, :]], axis=0)
        for dy in range(3)], axis=1).astype(f16)
    w1s = np.concatenate([w1f[:, dy, 2, :] for dy in range(3)],
                         axis=1).astype(f16)

    gw = np.asarray(inputs["g_w"], np.float32)[:, :, 0, 0]
    Ww = np.asarray(inputs["W_w"], np.float32)[:, :, 0, 0]
    wpk = np.zeros((2 * C, 1664), f16)
    wpk[:, 0:192] = w1p
    wpk[0:C, 192:384] = w1s
    wpk[0:C, 384:960] = conv_w(inputs["d2_w"])
    wpk[0:C, 960:1536] = conv_w(inputs["d3_w"])
    wpk[0:C, 1536:1568] = np.asarray(
        inputs["th_w"], np.float32)[:, :, 0, 0].T.astype(f16)
    wpk[0:C, 1568:1600] = (np.asarray(
        inputs["ph_w"], np.float32)[:, :, 0, 0].T * A_EXP).astype(f16)
    wpk[0:C, 1600:1664] = (Ww @ gw).T.astype(f16)
    base = {
        "xpad": xpad,
        "xpadB": xpadB,
        "wpk": wpk,
        "x16": x16p,
        "w1": conv_w(inputs["d1_w"]),
        "w2": conv_w(inputs["d2_w"]),
        "w3": conv_w(inputs["d3_w"]),
        "twT": np.ascontiguousarray(
            np.asarray(inputs["th_w"], np.float32)[:, :, 0, 0].T).astype(f16),
        "pwTA": np.ascontiguousarray(
            np.asarray(inputs["ph_w"], np.float32)[:, :, 0, 0].T
            * A_EXP).astype(f16),
        "ET": np.ascontiguousarray((Ww @ gw).T).astype(f16),
    }
    return [dict(base) for _ in range(NCORES)]


def kernel(**inputs):
    from concourse import bass_utils

    nc = get_program()
    in_maps = make_in_maps(inputs)
    res = bass_utils.run_bass_kernel_spmd(nc, in_maps,
                                          core_ids=list(range(NCORES)))
    # gather permuted chunks -> full permuted [C, N] -> un-permute
    outp = np.concatenate([res.results[k]["out"] for k in range(NCORES)],
                          axis=1)
    out = outp.reshape(C, 8, H, W // 8).transpose(0, 2, 3, 1).reshape(C, H, W)
    return out.reshape(1, C, H, W).astype(np.float32)
